# revision 23
# baseline (speedup 1.0000x reference)
"""Trainium2 Bass kernel for nn_CameraEstimator.

Computes, for each batch item b:
    camera[b] = einsum('chw,c->hw', x[b], W)          (C=256 contraction)
    out[b]    = nearest-rotation(camera[b])           (SVD u@vh + det reflection fix)

Pipeline per 128-row tile (32 tiles/core):
    SWDGE DMA with inline fp32->fp16 cast -> PE transpose (18x [128,128]) ->
    DVE/ACT copy PSUM->SBUF -> PE matmul vs masked-W (accumulate C) -> cam.
The SO(3) projection (scaled Newton polar + closed-form smallest-eigenvalue
reflection fix) runs on DVE/ACT in 3 chunks interleaved with the tile loop.
To shorten the serial dependency chain of the last chunk, the reflection's
P-factor is computed from the 2nd Newton iterate (numerically equivalent:
rel err 7.8e-4 either way) so the eigen/projector branch overlaps Newton
iterations 3-4.  All ACT transcendentals are Sqrt (one table set);
sin(acos(r)/3+pi/6) is a pair of factored quartics P(r)+sqrt(1-r)*Q(r).

Sharding: batch dim split evenly across 8 NeuronCores (data parallel), W
replicated.
"""

import numpy as np

import concourse.bacc as bacc
import concourse.bass as bass
import concourse.mybir as mybir
from concourse.bass_types import AP
from concourse.tile import TileContext
from concourse import bass_utils

F32 = mybir.dt.float32
F16 = mybir.dt.float16
ALU = mybir.AluOpType
ACT = mybir.ActivationFunctionType

B_FULL = 32768
C = 256
E = 9
N_CORES = 8
P = 128
B_LOCAL = B_FULL // N_CORES          # 4096
TPC = B_LOCAL // P                   # 32 matrices per partition
NCH = (C * E) // P                   # 18 chunks of 128 per tile

CHUNKS = [(0, 16), (16, 26), (26, 32)]
DVE_COPY_TILES = {1, 4, 7, 10, 13, 16, 19, 22, 25}  # PSUM copies on DVE
SPLIT_COPY_TILES = 28                 # tiles >= this split copies DVE/ACT
# x DMA grouping: small transfers first (fast pipeline start), then 4-tile
# groups (fewer transfers -> no completion-sem reuse throttling)
DMA_GROUPS = [1, 1, 2, 4, 4, 4, 4, 4, 4, 4]

# sin(acos(r)/3 + pi/6) ~= c4P*(r^2+pb1*r+pb0)(r^2+pb3*r+pb2)
#                        + sqrt(1-r)*c4Q*(...)
C4P = -0.00012669774781398082
PB = [(-10.373862952604547, -41.02339770448056),
      (-6.311076547771094, 83.61692215433715)]
C4Q = 1.3807082105156778e-05
QB = [(-46.779313658342474, 452.0278334014067),
      (2.6583141961744774, 69.12475662024718)]


def v(base: AP, off: int, *dims) -> AP:
    """Free-dim view of an SBUF tile AP: keep partition dim, set free dims."""
    return AP(base.tensor, base.offset + off,
              [list(base.ap[0])] + [[s, c] for (s, c) in dims])


def make_wm(W: np.ndarray) -> np.ndarray:
    """Masked-W moving operand, partition-major: wm[k, j*9+m] = fp16(W[c])
    where c=(128j+k)//9 if (128j+k)%9 == m else 0, so that
    xT16_j.T @ wm[:, 9j:9j+9] accumulates camera.  Layout [P, NCH*E] keeps
    the DMA contiguous per partition (tiny-descriptor DMAs stall the SDMA
    round-robin and starve the x stream)."""
    kidx = np.arange(C * E)
    wh = np.zeros((C * E, E), np.float32)
    wh[kidx, kidx % E] = W[kidx // E]
    wjkm = wh.astype(np.float16).reshape(NCH, P, E)
    return np.ascontiguousarray(wjkm.transpose(1, 0, 2).reshape(P, NCH * E))


def _emit(nc, tc, x_ap, wm_ap, idt_ap, y_ap):
    f32 = F32
    vec = nc.vector
    act = nc.scalar

    x_flat = x_ap.rearrange("b c h w -> b (c h w)")
    x_tiled = x_flat.rearrange("(p t) f -> p t f", p=P)
    y_flat = y_ap.rearrange("b h w -> b (h w)").rearrange("(p t) e -> p (t e)", p=P)

    NMAX = max(t1 - t0 for t0, t1 in CHUNKS)

    with tc.tile_pool(name="x16", bufs=8) as x16pool, \
         tc.tile_pool(name="tp", bufs=6, space="PSUM") as tpp, \
         tc.tile_pool(name="pcp", bufs=2, space="PSUM") as pcp, \
         tc.tile_pool(name="cam", bufs=2) as campool, \
         tc.tile_pool(name="ck", bufs=2) as ck, \
         tc.tile_pool(name="wk", bufs=1) as wp:

        idt = wp.tile([P, P], F16)
        nc.sync.dma_start(out=idt[:], in_=idt_ap)
        wm_sb = wp.tile([P, NCH * E], F16)
        nc.sync.dma_start(out=wm_sb[:], in_=wm_ap)

        _consts = {}

        def cb(val):
            if val not in _consts:
                ct = wp.tile([P, 1], f32, name=f"const{len(_consts)}")
                vec.memset(ct[:], float(val))
                _consts[val] = ct[:]
            return _consts[val]

        # ---------------- SO(3) projection for one chunk -------------------
        def emit_so3(ci, cam_c, n):
            last = (ci == len(CHUNKS) - 1)
            NE = n * E

            def mat(tile, off=0):
                return v(tile, off, (E, n), (3, 3), (1, 3))

            def flat(tile):
                return v(tile, 0, (1, NE))

            def row0(tile):
                return v(tile, 0, (E, n), (1, 3))

            def diag(tile):
                return v(tile, 0, (E, n), (4, 3))

            def pl(tile):
                return v(tile, 0, (1, n))

            def bc9(tile):
                return v(tile, 0, (1, n), (0, E))

            def bc3(tile):
                return v(tile, 0, (1, n), (0, 3))

            nm = f"c{ci}"

            def big(tag):
                return ck.tile([P, NMAX * E], f32, tag=tag, name=f"{tag}{nm}")

            def plane(tag):
                return ck.tile([P, NMAX], f32, tag=tag, name=f"{tag}{nm}")

            Ya, Yb, Yc = big("Ya"), big("Yb"), big("Yc")
            CfN, t1, t2 = big("CfN"), big("t1"), big("t2")
            CfR, ra, rb, rc = big("CfR"), big("ra"), big("rb"), big("rc")
            D = ck.tile([P, NMAX * 36], f32, tag="D", name=f"D{nm}")
            D2 = ck.tile([P, NMAX * 36], f32, tag="D2", name=f"D2{nm}")
            td = ck.tile([P, NMAX * 3], f32, tag="td", name=f"td{nm}")
            td2 = ck.tile([P, NMAX * 3], f32, tag="td2", name=f"td2{nm}")
            det = plane("det")
            det0 = plane("det0")
            rdet = plane("rdet")
            mi = plane("mi")
            u1, u2, u3 = plane("u1"), plane("u2"), plane("u3")
            c2, c1, c0 = plane("c2"), plane("c1"), plane("c0")
            q, r, p26, pp = plane("q"), plane("r"), plane("p26"), plane("pp")
            sq, ha, hb, hp = plane("sq"), plane("ha"), plane("hb"), plane("hp")
            s3, w1, plv = plane("s3"), plane("w1"), plane("plv")

            def build_D(Y, Dst):
                src = v(Y, 0, (E, n), (3, 3), (0, 2), (1, 3))
                eng = vec.tensor_copy if last else act.copy
                for off in (0, 18):
                    eng(v(Dst, off, (36, n), (6, 3), (3, 2), (1, 3)), src)

            def dblock(Dst, off):
                return v(Dst, off, (36, n), (6, 3), (1, 3))

            def cofactor(Y, out, Dst, ta, tb):
                build_D(Y, Dst)
                vec.tensor_tensor(mat(ta), dblock(Dst, 7), dblock(Dst, 14),
                                  ALU.mult)
                vec.tensor_tensor(mat(tb), dblock(Dst, 8), dblock(Dst, 13),
                                  ALU.mult)
                vec.tensor_tensor(mat(out), mat(ta), mat(tb), ALU.subtract)

            def det_of(Y, Cof, out, tdx):
                vec.tensor_tensor(v(tdx, 0, (3, n), (1, 3)), row0(Y), row0(Cof),
                                  ALU.mult)
                vec.tensor_reduce(pl(out), v(tdx, 0, (3, n), (1, 3)),
                                  mybir.AxisListType.X, ALU.add)

            def newton_iter(Y, Yn, it, scaled):
                cofactor(Y, CfN, D, t1, t2)
                det_of(Y, CfN, det, td)
                if it == 0:
                    act.copy(pl(det0), pl(det))
                vec.reciprocal(pl(rdet), pl(det))
                if scaled:
                    # mu = |det|^(-3/8); runs parallel with rdet
                    vec.tensor_tensor(pl(u1), pl(det), pl(det), ALU.mult)
                    act.activation(pl(u1), pl(u1), ACT.Sqrt, bias=cb(1e-35))
                    act.activation(pl(u1), pl(u1), ACT.Sqrt, bias=cb(0.0))
                    act.activation(pl(u2), pl(u1), ACT.Sqrt, bias=cb(0.0))
                    act.activation(pl(u3), pl(u2), ACT.Sqrt, bias=cb(0.0))
                    vec.tensor_tensor(pl(mi), pl(u2), pl(u3), ALU.mult)  # 1/mu
                    # s = 0.5*(1/mu)*(1/det);  mu = 1/(1/mu)
                    vec.scalar_tensor_tensor(pl(u2), pl(mi), 0.5, pl(rdet),
                                             ALU.mult, ALU.mult)
                    vec.reciprocal(pl(u1), pl(mi))
                    vec.tensor_tensor(flat(t1), flat(Y), bc9(u1), ALU.mult)
                    vec.tensor_tensor(flat(t2), flat(CfN), bc9(u2), ALU.mult)
                    vec.scalar_tensor_tensor(flat(Yn), flat(t1), 0.5, flat(t2),
                                             ALU.mult, ALU.add)
                else:
                    vec.scalar_tensor_tensor(flat(t2), flat(CfN), 0.5,
                                             bc9(rdet), ALU.mult, ALU.mult)
                    vec.scalar_tensor_tensor(flat(Yn), flat(Y), 0.5, flat(t2),
                                             ALU.mult, ALU.add)

            # Newton iters 1-2 (scaled)
            newton_iter(cam_c, Ya, 0, True)
            newton_iter(Ya, Yb, 1, True)

            # ---- reflection prep from Y2 (=Yb), concurrent with iters 3-4 --
            Pm = rb
            for k in range(3):
                a = v(Yb, 3 * k, (E, n), (1, 3), (0, 3))
                b = v(cam_c, 3 * k, (E, n), (0, 3), (1, 3))
                dst = (Pm, ra, CfR)[k]
                vec.tensor_tensor(mat(dst), a, b, ALU.mult)
            vec.tensor_tensor(mat(ra), mat(ra), mat(CfR), ALU.add)
            vec.tensor_tensor(mat(Pm), mat(Pm), mat(ra), ALU.add)

            cofactor(Pm, CfR, D2, ra, rc)
            vec.tensor_reduce(pl(c2), diag(Pm), mybir.AxisListType.X, ALU.add)
            vec.tensor_reduce(pl(c1), diag(CfR), mybir.AxisListType.X, ALU.add)
            det_of(Pm, CfR, c0, td2)

            vec.tensor_scalar(pl(q), pl(c2), 1.0 / 3.0, None, ALU.mult)
            vec.scalar_tensor_tensor(pl(p26), pl(c2), 1.0 / 9.0, pl(c2),
                                     ALU.mult, ALU.mult)
            vec.scalar_tensor_tensor(pl(p26), pl(c1), -1.0 / 3.0, pl(p26),
                                     ALU.mult, ALU.add)
            vec.tensor_scalar(pl(p26), pl(p26), 0.0, None, ALU.max)
            act.activation(pl(pp), pl(p26), ACT.Sqrt, bias=cb(1e-30))
            vec.scalar_tensor_tensor(pl(r), pl(c2), 2.0 / 9.0, pl(c2),
                                     ALU.mult, ALU.mult)
            vec.tensor_tensor(pl(r), pl(r), pl(c1), ALU.subtract)
            vec.tensor_tensor(pl(r), pl(r), pl(q), ALU.mult)
            vec.tensor_tensor(pl(r), pl(r), pl(c0), ALU.add)
            vec.scalar_tensor_tensor(pl(plv), pl(p26), 2.0, pl(pp),
                                     ALU.mult, ALU.mult)
            vec.tensor_scalar(pl(plv), pl(plv), 1e-30, None, ALU.add)
            vec.reciprocal(pl(plv), pl(plv))
            vec.tensor_tensor(pl(r), pl(r), pl(plv), ALU.mult)
            vec.tensor_scalar(pl(r), pl(r), -1.0, 1.0, ALU.max, ALU.min)
            act.activation(pl(sq), pl(r), ACT.Sqrt, scale=-1.0, bias=cb(1.0))
            vec.tensor_tensor(pl(u3), pl(r), pl(r), ALU.mult)   # r^2 (u3 safe:
            # newton u3 only used in scaled iters 1-2 which precede this)
            vec.scalar_tensor_tensor(pl(ha), pl(r), PB[0][0], pl(u3),
                                     ALU.mult, ALU.add)
            vec.tensor_scalar(pl(ha), pl(ha), PB[0][1], None, ALU.add)
            vec.scalar_tensor_tensor(pl(hb), pl(r), PB[1][0], pl(u3),
                                     ALU.mult, ALU.add)
            vec.tensor_scalar(pl(hb), pl(hb), PB[1][1], None, ALU.add)
            vec.scalar_tensor_tensor(pl(hp), pl(ha), C4P, pl(hb),
                                     ALU.mult, ALU.mult)
            vec.scalar_tensor_tensor(pl(ha), pl(r), QB[0][0], pl(u3),
                                     ALU.mult, ALU.add)
            vec.tensor_scalar(pl(ha), pl(ha), QB[0][1], None, ALU.add)
            vec.scalar_tensor_tensor(pl(hb), pl(r), QB[1][0], pl(u3),
                                     ALU.mult, ALU.add)
            vec.tensor_scalar(pl(hb), pl(hb), QB[1][1], None, ALU.add)
            vec.scalar_tensor_tensor(pl(ha), pl(ha), C4Q, pl(hb),
                                     ALU.mult, ALU.mult)
            vec.tensor_tensor(pl(ha), pl(ha), pl(sq), ALU.mult)
            vec.tensor_tensor(pl(hp), pl(hp), pl(ha), ALU.add)
            vec.scalar_tensor_tensor(pl(s3), pl(pp), -2.0, pl(hp),
                                     ALU.mult, ALU.mult)
            vec.tensor_tensor(pl(s3), pl(s3), pl(q), ALU.add)
            # Nadj = CP + s3*P + (s3^2 - s3*c2) I ; proj = Nadj/tr -> CfR
            vec.scalar_tensor_tensor(pl(w1), pl(c2), -1.0, pl(s3),
                                     ALU.mult, ALU.add)
            vec.tensor_tensor(pl(w1), pl(w1), pl(s3), ALU.mult)
            vec.tensor_tensor(flat(ra), flat(Pm), bc9(s3), ALU.mult)
            vec.tensor_tensor(flat(CfR), flat(CfR), flat(ra), ALU.add)
            vec.tensor_tensor(diag(CfR), diag(CfR), bc3(w1), ALU.add)
            vec.tensor_reduce(pl(plv), diag(CfR), mybir.AxisListType.X, ALU.add)
            vec.tensor_scalar(pl(plv), pl(plv), 1e-30, None, ALU.add)
            vec.reciprocal(pl(plv), pl(plv))
            vec.tensor_tensor(flat(CfR), flat(CfR), bc9(plv), ALU.mult)

            # Newton iters 3-4 (emitted after prep; scheduler overlaps)
            newton_iter(Yb, Yc, 2, True)
            newton_iter(Yc, Ya, 3, False)
            orth = Ya

            # corr = orth @ proj (tree), then R = orth - clamp(2*(det0<0)*corr)
            corr = rb  # Pm dead after Nadj
            for k in range(3):
                a = v(orth, k, (E, n), (3, 3), (0, 3))
                b = v(CfR, 3 * k, (E, n), (0, 3), (1, 3))
                dst = (corr, ra, t1)[k]
                vec.tensor_tensor(mat(dst), a, b, ALU.mult)
            vec.tensor_tensor(mat(ra), mat(ra), mat(t1), ALU.add)
            vec.tensor_tensor(mat(corr), mat(corr), mat(ra), ALU.add)
            vec.tensor_scalar(pl(plv), pl(det0), 0.0, 2.0, ALU.is_lt, ALU.mult)
            vec.tensor_tensor(flat(corr), flat(corr), bc9(plv), ALU.mult)
            vec.tensor_scalar(flat(corr), flat(corr), -2.0, 2.0, ALU.max,
                              ALU.min)
            vec.tensor_tensor(flat(t1), flat(orth), flat(corr), ALU.subtract)

            t0c = CHUNKS[ci][0]
            yv = AP(y_flat.tensor, y_flat.offset + t0c * E,
                    [list(y_flat.ap[0]), [1, NE]])
            nc.sync.dma_start(out=yv, in_=flat(t1))

        # ---------------- main tile loop -----------------------------------
        cam_c = None
        chunk_of = {}
        for ci, (t0, t1_) in enumerate(CHUNKS):
            for t in range(t0, t1_):
                chunk_of[t] = (ci, t0, t1_)

        grp_of = {}
        tg = 0
        for gsz in DMA_GROUPS:
            for t in range(tg, tg + gsz):
                grp_of[t] = (tg, gsz)
            tg += gsz

        xt16 = None
        for t in range(TPC):
            ci, t0, t1_ = chunk_of[t]
            if t == t0:
                cam_c = campool.tile([P, NMAX * E], f32, tag="cam",
                                     name=f"cam{ci}")
            g0, gsz = grp_of[t]
            if t == g0:
                xt16 = x16pool.tile([P, 4 * C * E], F16, tag="xt16",
                                    name=f"xt16_{t}")
                # SWDGE DMA with inline fp32->fp16 cast (read-bound on HBM)
                nc.gpsimd.dma_start(out=xt16[:, :gsz * C * E],
                                    in_=x_tiled[:, g0:g0 + gsz, :])
            toff = (t - g0) * C * E
            xT = x16pool.tile([P, C * E], F16, tag="xT", name=f"xT{t}")
            for g, (c0_, nch) in enumerate(((0, 8), (8, 8), (16, 2))):
                pt = tpp.tile([P, 1024], F16, tag="pt", name=f"pt{t}_{g}")
                for a in range(nch):
                    j = c0_ + a
                    nc.tensor.transpose(pt[:, P * a:P * (a + 1)],
                                        xt16[:, toff + P * j:toff + P * (j + 1)],
                                        idt[:])
                on_dve = (t in DVE_COPY_TILES) or \
                    (t >= SPLIT_COPY_TILES and g == 1)
                if on_dve:
                    # int32 reinterpret: bit-exact on DVE (ACT would round)
                    vec.tensor_copy(
                        xT[:, P * c0_:P * (c0_ + nch)].bitcast(mybir.dt.int32),
                        pt[:, :P * nch].bitcast(mybir.dt.int32))
                else:
                    act.copy(xT[:, P * c0_:P * (c0_ + nch)], pt[:, :P * nch])
            pc = pcp.tile([P, E], f32, tag="pc", name=f"pc{t}")
            for j in range(NCH):
                nc.tensor.matmul(pc[:], xT[:, P * j:P * (j + 1)],
                                 v(wm_sb, E * j, (1, E)),
                                 start=(j == 0), stop=(j == NCH - 1))
            if t >= SPLIT_COPY_TILES:
                vec.tensor_copy(v(cam_c, (t - t0) * E, (1, E)), pc[:])
            else:
                act.copy(v(cam_c, (t - t0) * E, (1, E)), pc[:])
            if t == t1_ - 1:
                emit_so3(ci, cam_c, t1_ - t0)


def build(b_local=B_LOCAL):
    nc = bacc.Bacc("TRN2", target_bir_lowering=False, debug=False)
    x = nc.dram_tensor("x", [b_local, C, 3, 3], F32, kind="ExternalInput")
    wm = nc.dram_tensor("wm", [P, NCH * E], F16, kind="ExternalInput")
    idt = nc.dram_tensor("idt", [P, P], F16, kind="ExternalInput")
    y = nc.dram_tensor("y", [b_local, 3, 3], F32, kind="ExternalOutput")
    with TileContext(nc) as tc:
        _emit(nc, tc, x.ap(), wm.ap(), idt.ap(), y.ap())
    nc.compile()
    return nc


_NC_CACHE = {}


def kernel(x: np.ndarray, W: np.ndarray) -> np.ndarray:
    assert x.shape == (B_FULL, C, 3, 3) and W.shape == (C,)
    if "nc" not in _NC_CACHE:
        _NC_CACHE["nc"] = build()
    nc = _NC_CACHE["nc"]
    xs = np.ascontiguousarray(x.reshape(N_CORES, B_LOCAL, C, 3, 3))
    wmn = make_wm(np.asarray(W, dtype=np.float32))
    idn = np.eye(P, dtype=np.float16)
    in_maps = [{"x": xs[i], "wm": wmn, "idt": idn} for i in range(N_CORES)]
    res = bass_utils.run_bass_kernel_spmd(nc, in_maps, core_ids=list(range(N_CORES)))
    return np.concatenate([r["y"] for r in res.results], axis=0)


if __name__ == "__main__":
    rng = np.random.default_rng(0)
    x = rng.standard_normal((B_FULL, C, 3, 3), dtype=np.float32)
    W = (rng.standard_normal(C, dtype=np.float32) / np.sqrt(C)).astype(np.float32)
    out = kernel(x=x, W=W)
    print(out.shape, out.dtype)


# revision 25
# speedup vs baseline: 1.1015x; 1.1015x over previous
"""Trainium2 Bass kernel for nn_CameraEstimator.

Computes, for each batch item b:
    camera[b] = einsum('chw,c->hw', x[b], W)          (C=256 contraction)
    out[b]    = nearest-rotation(camera[b])           (SVD u@vh + det reflection fix)

Pipeline per 128-row tile (32 tiles/core):
    SWDGE DMA with inline fp32->fp16 cast -> PE transpose (18x [128,128]) ->
    DVE/ACT copy PSUM->SBUF -> PE matmul vs masked-W (accumulate C) -> cam.
The SO(3) projection (scaled Newton polar + closed-form smallest-eigenvalue
reflection fix) runs on DVE/ACT in 3 chunks interleaved with the tile loop.
To shorten the serial dependency chain of the last chunk, the reflection's
P-factor is computed from the 2nd Newton iterate (numerically equivalent:
rel err 7.8e-4 either way) so the eigen/projector branch overlaps Newton
iterations 3-4.  All ACT transcendentals are Sqrt (one table set);
sin(acos(r)/3+pi/6) is a pair of factored quartics P(r)+sqrt(1-r)*Q(r).

Sharding: batch dim split evenly across 8 NeuronCores (data parallel), W
replicated.
"""

import numpy as np

import concourse.bacc as bacc
import concourse.bass as bass
import concourse.mybir as mybir
from concourse.bass_types import AP
from concourse.tile import TileContext
from concourse import bass_utils

F32 = mybir.dt.float32
F16 = mybir.dt.float16
ALU = mybir.AluOpType
ACT = mybir.ActivationFunctionType

B_FULL = 32768
C = 256
E = 9
N_CORES = 8
P = 128
B_LOCAL = B_FULL // N_CORES          # 4096
TPC = B_LOCAL // P                   # 32 matrices per partition
NCH = (C * E) // P                   # 18 chunks of 128 per tile

CHUNKS = [(0, 16), (16, 26), (26, 32)]
DVE_COPY_TILES = {1, 4, 7, 10, 13, 16, 19, 22, 25}  # PSUM copies on DVE
SPLIT_COPY_TILES = 28                 # tiles >= this split copies DVE/ACT
# x DMA grouping: small transfers first (fast pipeline start), then 4-tile
# groups (fewer transfers -> no completion-sem reuse throttling)
DMA_GROUPS = [1, 1] + [2] * 15

# sin(acos(r)/3 + pi/6) ~= c4P*(r^2+pb1*r+pb0)(r^2+pb3*r+pb2)
#                        + sqrt(1-r)*c4Q*(...)
C4P = -0.00012669774781398082
PB = [(-10.373862952604547, -41.02339770448056),
      (-6.311076547771094, 83.61692215433715)]
C4Q = 1.3807082105156778e-05
QB = [(-46.779313658342474, 452.0278334014067),
      (2.6583141961744774, 69.12475662024718)]


def v(base: AP, off: int, *dims) -> AP:
    """Free-dim view of an SBUF tile AP: keep partition dim, set free dims."""
    return AP(base.tensor, base.offset + off,
              [list(base.ap[0])] + [[s, c] for (s, c) in dims])


def make_wm(W: np.ndarray) -> np.ndarray:
    """Masked-W moving operand, partition-major: wm[k, j*9+m] = fp16(W[c])
    where c=(128j+k)//9 if (128j+k)%9 == m else 0, so that
    xT16_j.T @ wm[:, 9j:9j+9] accumulates camera.  Layout [P, NCH*E] keeps
    the DMA contiguous per partition (tiny-descriptor DMAs stall the SDMA
    round-robin and starve the x stream)."""
    kidx = np.arange(C * E)
    wh = np.zeros((C * E, E), np.float32)
    wh[kidx, kidx % E] = W[kidx // E]
    wjkm = wh.astype(np.float16).reshape(NCH, P, E)
    return np.ascontiguousarray(wjkm.transpose(1, 0, 2).reshape(P, NCH * E))


def _emit(nc, tc, x_ap, wm_ap, idt_ap, y_ap):
    f32 = F32
    vec = nc.vector
    act = nc.scalar

    x_flat = x_ap.rearrange("b c h w -> b (c h w)")
    x_tiled = x_flat.rearrange("(p t) f -> p t f", p=P)
    y_flat = y_ap.rearrange("b h w -> b (h w)").rearrange("(p t) e -> p (t e)", p=P)

    NMAX = max(t1 - t0 for t0, t1 in CHUNKS)

    with tc.tile_pool(name="x16", bufs=8) as x16pool, \
         tc.tile_pool(name="tp", bufs=6, space="PSUM") as tpp, \
         tc.tile_pool(name="pcp", bufs=2, space="PSUM") as pcp, \
         tc.tile_pool(name="cam", bufs=2) as campool, \
         tc.tile_pool(name="ck", bufs=2) as ck, \
         tc.tile_pool(name="wk", bufs=1) as wp:

        idt = wp.tile([P, P], F16)
        nc.sync.dma_start(out=idt[:], in_=idt_ap)
        wm_sb = wp.tile([P, NCH * E], F16)
        nc.sync.dma_start(out=wm_sb[:], in_=wm_ap)

        _consts = {}

        def cb(val):
            if val not in _consts:
                ct = wp.tile([P, 1], f32, name=f"const{len(_consts)}")
                vec.memset(ct[:], float(val))
                _consts[val] = ct[:]
            return _consts[val]

        # ---------------- SO(3) projection for one chunk -------------------
        def emit_so3(ci, cam_c, n):
            last = (ci == len(CHUNKS) - 1)
            NE = n * E

            def mat(tile, off=0):
                return v(tile, off, (E, n), (3, 3), (1, 3))

            def flat(tile):
                return v(tile, 0, (1, NE))

            def row0(tile):
                return v(tile, 0, (E, n), (1, 3))

            def diag(tile):
                return v(tile, 0, (E, n), (4, 3))

            def pl(tile):
                return v(tile, 0, (1, n))

            def bc9(tile):
                return v(tile, 0, (1, n), (0, E))

            def bc3(tile):
                return v(tile, 0, (1, n), (0, 3))

            nm = f"c{ci}"

            def big(tag):
                return ck.tile([P, NMAX * E], f32, tag=tag, name=f"{tag}{nm}")

            def plane(tag):
                return ck.tile([P, NMAX], f32, tag=tag, name=f"{tag}{nm}")

            Ya, Yb, Yc = big("Ya"), big("Yb"), big("Yc")
            CfN, t1, t2 = big("CfN"), big("t1"), big("t2")
            CfR, ra, rb, rc = big("CfR"), big("ra"), big("rb"), big("rc")
            D = ck.tile([P, NMAX * 36], f32, tag="D", name=f"D{nm}")
            D2 = ck.tile([P, NMAX * 36], f32, tag="D2", name=f"D2{nm}")
            td = ck.tile([P, NMAX * 3], f32, tag="td", name=f"td{nm}")
            td2 = ck.tile([P, NMAX * 3], f32, tag="td2", name=f"td2{nm}")
            det = plane("det")
            det0 = plane("det0")
            rdet = plane("rdet")
            mi = plane("mi")
            u1, u2, u3 = plane("u1"), plane("u2"), plane("u3")
            c2, c1, c0 = plane("c2"), plane("c1"), plane("c0")
            q, r, p26, pp = plane("q"), plane("r"), plane("p26"), plane("pp")
            sq, ha, hb, hp = plane("sq"), plane("ha"), plane("hb"), plane("hp")
            s3, w1, plv = plane("s3"), plane("w1"), plane("plv")

            def build_D(Y, Dst):
                src = v(Y, 0, (E, n), (3, 3), (0, 2), (1, 3))
                eng = vec.tensor_copy if last else act.copy
                for off in (0, 18):
                    eng(v(Dst, off, (36, n), (6, 3), (3, 2), (1, 3)), src)

            def dblock(Dst, off):
                return v(Dst, off, (36, n), (6, 3), (1, 3))

            def cofactor(Y, out, Dst, ta, tb):
                build_D(Y, Dst)
                vec.tensor_tensor(mat(ta), dblock(Dst, 7), dblock(Dst, 14),
                                  ALU.mult)
                vec.tensor_tensor(mat(tb), dblock(Dst, 8), dblock(Dst, 13),
                                  ALU.mult)
                vec.tensor_tensor(mat(out), mat(ta), mat(tb), ALU.subtract)

            def det_of(Y, Cof, out, tdx):
                vec.tensor_tensor(v(tdx, 0, (3, n), (1, 3)), row0(Y), row0(Cof),
                                  ALU.mult)
                vec.tensor_reduce(pl(out), v(tdx, 0, (3, n), (1, 3)),
                                  mybir.AxisListType.X, ALU.add)

            def newton_iter(Y, Yn, it, scaled):
                cofactor(Y, CfN, D, t1, t2)
                det_of(Y, CfN, det, td)
                if it == 0:
                    act.copy(pl(det0), pl(det))
                vec.reciprocal(pl(rdet), pl(det))
                if scaled:
                    # mu = |det|^(-3/8); runs parallel with rdet
                    vec.tensor_tensor(pl(u1), pl(det), pl(det), ALU.mult)
                    act.activation(pl(u1), pl(u1), ACT.Sqrt, bias=cb(1e-35))
                    act.activation(pl(u1), pl(u1), ACT.Sqrt, bias=cb(0.0))
                    act.activation(pl(u2), pl(u1), ACT.Sqrt, bias=cb(0.0))
                    act.activation(pl(u3), pl(u2), ACT.Sqrt, bias=cb(0.0))
                    vec.tensor_tensor(pl(mi), pl(u2), pl(u3), ALU.mult)  # 1/mu
                    # s = 0.5*(1/mu)*(1/det);  mu = 1/(1/mu)
                    vec.scalar_tensor_tensor(pl(u2), pl(mi), 0.5, pl(rdet),
                                             ALU.mult, ALU.mult)
                    vec.reciprocal(pl(u1), pl(mi))
                    vec.tensor_tensor(flat(t1), flat(Y), bc9(u1), ALU.mult)
                    vec.tensor_tensor(flat(t2), flat(CfN), bc9(u2), ALU.mult)
                    vec.scalar_tensor_tensor(flat(Yn), flat(t1), 0.5, flat(t2),
                                             ALU.mult, ALU.add)
                else:
                    vec.scalar_tensor_tensor(flat(t2), flat(CfN), 0.5,
                                             bc9(rdet), ALU.mult, ALU.mult)
                    vec.scalar_tensor_tensor(flat(Yn), flat(Y), 0.5, flat(t2),
                                             ALU.mult, ALU.add)

            # Newton iters 1-2 (scaled)
            newton_iter(cam_c, Ya, 0, True)
            newton_iter(Ya, Yb, 1, True)

            # ---- reflection prep from Y2 (=Yb), concurrent with iters 3-4 --
            Pm = rb
            for k in range(3):
                a = v(Yb, 3 * k, (E, n), (1, 3), (0, 3))
                b = v(cam_c, 3 * k, (E, n), (0, 3), (1, 3))
                dst = (Pm, ra, CfR)[k]
                vec.tensor_tensor(mat(dst), a, b, ALU.mult)
            vec.tensor_tensor(mat(ra), mat(ra), mat(CfR), ALU.add)
            vec.tensor_tensor(mat(Pm), mat(Pm), mat(ra), ALU.add)

            cofactor(Pm, CfR, D2, ra, rc)
            vec.tensor_reduce(pl(c2), diag(Pm), mybir.AxisListType.X, ALU.add)
            vec.tensor_reduce(pl(c1), diag(CfR), mybir.AxisListType.X, ALU.add)
            det_of(Pm, CfR, c0, td2)

            vec.tensor_scalar(pl(q), pl(c2), 1.0 / 3.0, None, ALU.mult)
            vec.scalar_tensor_tensor(pl(p26), pl(c2), 1.0 / 9.0, pl(c2),
                                     ALU.mult, ALU.mult)
            vec.scalar_tensor_tensor(pl(p26), pl(c1), -1.0 / 3.0, pl(p26),
                                     ALU.mult, ALU.add)
            vec.tensor_scalar(pl(p26), pl(p26), 0.0, None, ALU.max)
            act.activation(pl(pp), pl(p26), ACT.Sqrt, bias=cb(1e-30))
            vec.scalar_tensor_tensor(pl(r), pl(c2), 2.0 / 9.0, pl(c2),
                                     ALU.mult, ALU.mult)
            vec.tensor_tensor(pl(r), pl(r), pl(c1), ALU.subtract)
            vec.tensor_tensor(pl(r), pl(r), pl(q), ALU.mult)
            vec.tensor_tensor(pl(r), pl(r), pl(c0), ALU.add)
            vec.scalar_tensor_tensor(pl(plv), pl(p26), 2.0, pl(pp),
                                     ALU.mult, ALU.mult)
            vec.tensor_scalar(pl(plv), pl(plv), 1e-30, None, ALU.add)
            vec.reciprocal(pl(plv), pl(plv))
            vec.tensor_tensor(pl(r), pl(r), pl(plv), ALU.mult)
            vec.tensor_scalar(pl(r), pl(r), -1.0, 1.0, ALU.max, ALU.min)
            act.activation(pl(sq), pl(r), ACT.Sqrt, scale=-1.0, bias=cb(1.0))
            vec.tensor_tensor(pl(u3), pl(r), pl(r), ALU.mult)   # r^2 (u3 safe:
            # newton u3 only used in scaled iters 1-2 which precede this)
            vec.scalar_tensor_tensor(pl(ha), pl(r), PB[0][0], pl(u3),
                                     ALU.mult, ALU.add)
            vec.tensor_scalar(pl(ha), pl(ha), PB[0][1], None, ALU.add)
            vec.scalar_tensor_tensor(pl(hb), pl(r), PB[1][0], pl(u3),
                                     ALU.mult, ALU.add)
            vec.tensor_scalar(pl(hb), pl(hb), PB[1][1], None, ALU.add)
            vec.scalar_tensor_tensor(pl(hp), pl(ha), C4P, pl(hb),
                                     ALU.mult, ALU.mult)
            vec.scalar_tensor_tensor(pl(ha), pl(r), QB[0][0], pl(u3),
                                     ALU.mult, ALU.add)
            vec.tensor_scalar(pl(ha), pl(ha), QB[0][1], None, ALU.add)
            vec.scalar_tensor_tensor(pl(hb), pl(r), QB[1][0], pl(u3),
                                     ALU.mult, ALU.add)
            vec.tensor_scalar(pl(hb), pl(hb), QB[1][1], None, ALU.add)
            vec.scalar_tensor_tensor(pl(ha), pl(ha), C4Q, pl(hb),
                                     ALU.mult, ALU.mult)
            vec.tensor_tensor(pl(ha), pl(ha), pl(sq), ALU.mult)
            vec.tensor_tensor(pl(hp), pl(hp), pl(ha), ALU.add)
            vec.scalar_tensor_tensor(pl(s3), pl(pp), -2.0, pl(hp),
                                     ALU.mult, ALU.mult)
            vec.tensor_tensor(pl(s3), pl(s3), pl(q), ALU.add)
            # Nadj = CP + s3*P + (s3^2 - s3*c2) I ; proj = Nadj/tr -> CfR
            vec.scalar_tensor_tensor(pl(w1), pl(c2), -1.0, pl(s3),
                                     ALU.mult, ALU.add)
            vec.tensor_tensor(pl(w1), pl(w1), pl(s3), ALU.mult)
            vec.tensor_tensor(flat(ra), flat(Pm), bc9(s3), ALU.mult)
            vec.tensor_tensor(flat(CfR), flat(CfR), flat(ra), ALU.add)
            vec.tensor_tensor(diag(CfR), diag(CfR), bc3(w1), ALU.add)
            vec.tensor_reduce(pl(plv), diag(CfR), mybir.AxisListType.X, ALU.add)
            vec.tensor_scalar(pl(plv), pl(plv), 1e-30, None, ALU.add)
            vec.reciprocal(pl(plv), pl(plv))
            vec.tensor_tensor(flat(CfR), flat(CfR), bc9(plv), ALU.mult)

            # Newton iters 3-4 (emitted after prep; scheduler overlaps)
            newton_iter(Yb, Yc, 2, True)
            newton_iter(Yc, Ya, 3, False)
            orth = Ya

            # corr = orth @ proj (tree), then R = orth - clamp(2*(det0<0)*corr)
            corr = rb  # Pm dead after Nadj
            for k in range(3):
                a = v(orth, k, (E, n), (3, 3), (0, 3))
                b = v(CfR, 3 * k, (E, n), (0, 3), (1, 3))
                dst = (corr, ra, t1)[k]
                vec.tensor_tensor(mat(dst), a, b, ALU.mult)
            vec.tensor_tensor(mat(ra), mat(ra), mat(t1), ALU.add)
            vec.tensor_tensor(mat(corr), mat(corr), mat(ra), ALU.add)
            vec.tensor_scalar(pl(plv), pl(det0), 0.0, 2.0, ALU.is_lt, ALU.mult)
            vec.tensor_tensor(flat(corr), flat(corr), bc9(plv), ALU.mult)
            vec.tensor_scalar(flat(corr), flat(corr), -2.0, 2.0, ALU.max,
                              ALU.min)
            vec.tensor_tensor(flat(t1), flat(orth), flat(corr), ALU.subtract)

            t0c = CHUNKS[ci][0]
            yv = AP(y_flat.tensor, y_flat.offset + t0c * E,
                    [list(y_flat.ap[0]), [1, NE]])
            nc.sync.dma_start(out=yv, in_=flat(t1))

        # ---------------- main tile loop -----------------------------------
        cam_c = None
        chunk_of = {}
        for ci, (t0, t1_) in enumerate(CHUNKS):
            for t in range(t0, t1_):
                chunk_of[t] = (ci, t0, t1_)

        grp_of = {}
        tg = 0
        for gsz in DMA_GROUPS:
            for t in range(tg, tg + gsz):
                grp_of[t] = (tg, gsz)
            tg += gsz

        xt16 = None
        for t in range(TPC):
            ci, t0, t1_ = chunk_of[t]
            if t == t0:
                cam_c = campool.tile([P, NMAX * E], f32, tag="cam",
                                     name=f"cam{ci}")
            g0, gsz = grp_of[t]
            if t == g0:
                xt16 = x16pool.tile([P, 2 * C * E], F16, tag="xt16",
                                    name=f"xt16_{t}")
                # SWDGE DMA with inline fp32->fp16 cast (read-bound on HBM)
                nc.gpsimd.dma_start(out=xt16[:, :gsz * C * E],
                                    in_=x_tiled[:, g0:g0 + gsz, :])
            toff = (t - g0) * C * E
            xT = x16pool.tile([P, C * E], F16, tag="xT", name=f"xT{t}")
            for g, (c0_, nch) in enumerate(((0, 8), (8, 8), (16, 2))):
                pt = tpp.tile([P, 1024], F16, tag="pt", name=f"pt{t}_{g}")
                for a in range(nch):
                    j = c0_ + a
                    nc.tensor.transpose(pt[:, P * a:P * (a + 1)],
                                        xt16[:, toff + P * j:toff + P * (j + 1)],
                                        idt[:])
                on_dve = (t in DVE_COPY_TILES) or \
                    (t >= SPLIT_COPY_TILES and g == 1)
                if on_dve:
                    # int32 reinterpret: bit-exact on DVE (ACT would round)
                    vec.tensor_copy(
                        xT[:, P * c0_:P * (c0_ + nch)].bitcast(mybir.dt.int32),
                        pt[:, :P * nch].bitcast(mybir.dt.int32))
                else:
                    act.copy(xT[:, P * c0_:P * (c0_ + nch)], pt[:, :P * nch])
            pc = pcp.tile([P, E], f32, tag="pc", name=f"pc{t}")
            for j in range(NCH):
                nc.tensor.matmul(pc[:], xT[:, P * j:P * (j + 1)],
                                 v(wm_sb, E * j, (1, E)),
                                 start=(j == 0), stop=(j == NCH - 1))
            if t >= SPLIT_COPY_TILES:
                vec.tensor_copy(v(cam_c, (t - t0) * E, (1, E)), pc[:])
            else:
                act.copy(v(cam_c, (t - t0) * E, (1, E)), pc[:])
            if t == t1_ - 1:
                emit_so3(ci, cam_c, t1_ - t0)


def build(b_local=B_LOCAL):
    nc = bacc.Bacc("TRN2", target_bir_lowering=False, debug=False)
    x = nc.dram_tensor("x", [b_local, C, 3, 3], F32, kind="ExternalInput")
    wm = nc.dram_tensor("wm", [P, NCH * E], F16, kind="ExternalInput")
    idt = nc.dram_tensor("idt", [P, P], F16, kind="ExternalInput")
    y = nc.dram_tensor("y", [b_local, 3, 3], F32, kind="ExternalOutput")
    with TileContext(nc) as tc:
        _emit(nc, tc, x.ap(), wm.ap(), idt.ap(), y.ap())
    nc.compile()
    return nc


_NC_CACHE = {}


def kernel(x: np.ndarray, W: np.ndarray) -> np.ndarray:
    assert x.shape == (B_FULL, C, 3, 3) and W.shape == (C,)
    if "nc" not in _NC_CACHE:
        _NC_CACHE["nc"] = build()
    nc = _NC_CACHE["nc"]
    xs = np.ascontiguousarray(x.reshape(N_CORES, B_LOCAL, C, 3, 3))
    wmn = make_wm(np.asarray(W, dtype=np.float32))
    idn = np.eye(P, dtype=np.float16)
    in_maps = [{"x": xs[i], "wm": wmn, "idt": idn} for i in range(N_CORES)]
    res = bass_utils.run_bass_kernel_spmd(nc, in_maps, core_ids=list(range(N_CORES)))
    return np.concatenate([r["y"] for r in res.results], axis=0)


if __name__ == "__main__":
    rng = np.random.default_rng(0)
    x = rng.standard_normal((B_FULL, C, 3, 3), dtype=np.float32)
    W = (rng.standard_normal(C, dtype=np.float32) / np.sqrt(C)).astype(np.float32)
    out = kernel(x=x, W=W)
    print(out.shape, out.dtype)
